# revision 25
# baseline (speedup 1.0000x reference)
"""Trainium2 Bass kernel for ConduitHydrology (GNN message passing on a
1500x1500 raster grid).

The mesh is the fixed 2D raster built by the reference: every segment_sum
over head/tail collapses into a 5-point stencil.  The residual is
  res = dis - flux,  flux = OPEN*cs^1.25*|g|^-0.5*g  (|flux| <~ 2e-4)
so the residual is dominated by `dis`; every other input only feeds the
tiny flux term, which lets the whole stencil+conduit pipeline run in bf16
with enormous margin vs the 2e-2 tolerance (dis itself stays f32).

Sharding: 2x4 grid of cores, each owns a 750x375 node block, split into
6 row-bands of 125 rows.  All cross-partition (vertical) stencil work is
done on the otherwise-idle PE as shift-matrix matmuls accumulating in
PSUM (gradient: Wver*neC + Wp1*neE + Wm1*neW + I*geo; velocity:
Kvv*vv + I*vhW + I*vhC), with constants folded into host-scaled inputs:
  A   = ne * (kappa/(4L))        [ne = where(stat, over, eff), edge-pad]
  G   = psum_g = stencil(A)+geo*kappa = kappa*gradient, kappa=OPEN/SCALE
  C   = |psum_v| = cav/SCALE     [vh, vv scaled by STEP/(4*SEC*SCALE)]
  ncs = (dis*G + C)/(C + c3*A^3) = cs/SCALE,   c3 = CLOSURE/(kappa/(4L))^3
  flux= ncs_c^1.25 * G * 1/sqrt(s*|G|),        s = Phi^-2,
        Phi = OPEN*SCALE^1.25/sqrt(kappa)
Global frame nodes (link_count != 4) are fixed up exactly on the host
(5996 of 2.25M nodes).
"""

import sys

import numpy as np

if "/opt/trn_rl_repo" not in sys.path:
    sys.path.insert(0, "/opt/trn_rl_repo")

import ml_dtypes

BF16 = ml_dtypes.bfloat16
FP8 = (ml_dtypes.float8_e4m3fn if hasattr(ml_dtypes, "float8_e4m3fn")
       else ml_dtypes.float8_e4m3)

# ---- problem constants (from the reference model) ----
NROWS, NCOLS = 1500, 1500
OPENING_COEFF = 1.3455e-09
CLOSURE_COEFF = 7.11e-24
FLOW_EXP = 1.25
STEP_HEIGHT = 0.03
SCALE_CUTOFF = 5.74
N_EXP = 3
SEC_PER_A = 31556926.0
DX = 100.0

# ---- folded constants ----
ALPHA = 1.0 / (4.0 * DX)                     # 1/(L*cnt), interior cnt=4
KAPPA = OPENING_COEFF / SCALE_CUTOFF         # gradient scale
AK = ALPHA * KAPPA                           # ne scale
BETA = STEP_HEIGHT / (4.0 * SEC_PER_A * SCALE_CUTOFF)  # velocity scale
C3 = CLOSURE_COEFF / (AK ** 3)               # conduit denominator scale
PHI = OPENING_COEFF * SCALE_CUTOFF ** 1.25 / np.sqrt(KAPPA)
S_ARS = 1.0 / (PHI * PHI)                    # Abs_reciprocal_sqrt scale
NCS_CLAMP = 1e-6 / SCALE_CUTOFF              # conduit-size clamp on ncs
PHI08 = PHI ** 0.8                           # folds Phi^2 into ncs^2.5
SNE = 2.0 ** 21                              # fp8 scale for ne/geo/ne3
SV8 = 2.0 ** 26                              # fp8 scale for vh/vv

# ---- sharding geometry: 4x2 grid of cores ----
# 750-wide rows keep fp8 DMA descriptors >= 512B (full DMA rate)
CI, CJ = 4, 2
BR, BC = NROWS // CI, NCOLS // CJ            # 375 x 750 per core
NB = 3                                       # row bands per core
PB = BR // NB                                # 125 rows per band
HC = BC // 2                                 # 375: matmul col-half (PSUM bank)
WNE = BC + 2                                 # 752 ne cols (with halo)

_NC_CACHE = {}


def _patch_tile_drain():
    """The end-of-kernel Drain that Tile emits carries one sync-wait per
    outstanding semaphore; this stack's codegen rejects instructions with
    more than a handful of waits.  Split the collector into one NOP per
    proc, each carrying exactly one wait (the sync queue is in-order, so
    this is equivalent)."""
    from concourse import tile as _tile
    from concourse.vector_clock import ScopedClock, VectorClock

    if getattr(_tile.TileContext, "_drain_patched", False):
        return

    def _drain_and_barrier(self, tick_clock, wait_clock):
        gc = tick_clock.global_clock
        n = len(gc)
        for proc in range(n):
            t = gc[proc]
            if t <= 0:
                continue
            nop = self.nc.sync.nop()
            vc = VectorClock([0] * n)
            vc.require_at_least(proc, t)
            wait_clock.add_sem_waits(nop.ins, ScopedClock({None: vc}))
        self.nc.sync.drain()
        self.nc.all_engine_barrier()
        assert self.sems is not None
        popped = self.nc._tile_sem_poison_stack.pop()
        assert popped is self._sem_poison
        self.nc.clear_and_free_semaphores(list(self.sems.allocated().values()))
        self.nc.all_engine_barrier()

    _tile.TileContext._drain_and_barrier = _drain_and_barrier
    _tile.TileContext._drain_patched = True


def _build_nc():
    import concourse.bass as bass
    import concourse.mybir as mybir
    from concourse import bacc
    from concourse.tile import TileContext

    _patch_tile_drain()

    f32 = mybir.dt.float32
    bf16 = mybir.dt.bfloat16
    f8 = mybir.dt.float8e4
    Alu = mybir.AluOpType
    Act = mybir.ActivationFunctionType

    nc = bass.Bass()

    ne_d = nc.dram_tensor("ne", [BR + 2, WNE], f8, kind="ExternalInput")
    ne3_d = nc.dram_tensor("ne3", [BR, BC], bf16, kind="ExternalInput")
    dis_d = nc.dram_tensor("dis", [BR, BC], bf16, kind="ExternalInput")
    geo_d = nc.dram_tensor("geo", [BR, BC], f8, kind="ExternalInput")
    vh_d = nc.dram_tensor("vh", [BR, BC + 1], f8, kind="ExternalInput")
    vv_d = nc.dram_tensor("vv", [BR + 1, BC], f8, kind="ExternalInput")
    wf_d = nc.dram_tensor("wf", [127, 5 * 128], f8, kind="ExternalInput")
    out_d = nc.dram_tensor("res", [BR, BC], bf16, kind="ExternalOutput")

    with TileContext(nc) as tc:
        with tc.tile_pool(name="p", bufs=1) as pool, \
                tc.psum_pool(name="pp", bufs=1) as ppool, \
                nc.allow_low_precision(
                    reason="flux term is <1e-4 of the residual; bf16/fp8 "
                    "error is far inside the 2e-2 tolerance"):
            t_ne = pool.tile([127, NB, WNE], f8, tag="ne")
            t_ne3 = pool.tile([125, NB, BC], bf16, tag="ne3")
            t_dis = pool.tile([125, NB, BC], bf16, tag="dis")
            t_geo = pool.tile([125, NB, BC], f8, tag="geo")
            t_vh = pool.tile([125, NB, BC + 1], f8, tag="vh")
            t_vv = pool.tile([126, NB, BC], f8, tag="vv")
            t_w = pool.tile([127, 5, 128], f8, tag="wf")

            # loads: band 0's inputs stream first (small dedicated DMAs)
            # so its conduit chain starts ~4us earlier; bands 1-2 follow as
            # bulk transfers.  dis/ne3 stay per-band.
            nc.sync.dma_start(out=t_w[:], in_=wf_d[:])
            nc.sync.dma_start(
                out=t_ne[:, 0, :],
                in_=bass.AP(ne_d[:].tensor, 0, [[WNE, 127], [1, WNE]]))
            nc.sync.dma_start(
                out=t_geo[:, 0, :],
                in_=bass.AP(geo_d[:].tensor, 0, [[BC, 125], [1, BC]]))
            nc.sync.dma_start(
                out=t_vh[:, 0, :],
                in_=bass.AP(vh_d[:].tensor, 0, [[BC + 1, 125], [1, BC + 1]]))
            nc.sync.dma_start(
                out=t_vv[:, 0, :],
                in_=bass.AP(vv_d[:].tensor, 0, [[BC, 126], [1, BC]]))
            nc.sync.dma_start(
                out=t_dis[:, 0, :],
                in_=bass.AP(dis_d[:].tensor, 0, [[BC, 125], [1, BC]]))
            nc.sync.dma_start(
                out=t_ne3[:, 0, :],
                in_=bass.AP(ne3_d[:].tensor, 0, [[BC, 125], [1, BC]]))
            nc.sync.dma_start(
                out=t_ne[:, 1:NB, :],
                in_=bass.AP(ne_d[:].tensor, PB * WNE,
                            [[WNE, 127], [PB * WNE, NB - 1], [1, WNE]]))
            nc.sync.dma_start(
                out=t_geo[:, 1:NB, :],
                in_=bass.AP(geo_d[:].tensor, PB * BC,
                            [[BC, 125], [PB * BC, NB - 1], [1, BC]]))
            nc.sync.dma_start(
                out=t_vh[:, 1:NB, :],
                in_=bass.AP(vh_d[:].tensor, PB * (BC + 1),
                            [[BC + 1, 125], [PB * (BC + 1), NB - 1],
                             [1, BC + 1]]))
            nc.sync.dma_start(
                out=t_vv[:, 1:NB, :],
                in_=bass.AP(vv_d[:].tensor, PB * BC,
                            [[BC, 126], [PB * BC, NB - 1], [1, BC]]))
            for b in range(1, NB):
                nc.sync.dma_start(
                    out=t_dis[:, b, :],
                    in_=bass.AP(dis_d[:].tensor, b * PB * BC,
                                [[BC, 125], [1, BC]]))
                nc.sync.dma_start(
                    out=t_ne3[:, b, :],
                    in_=bass.AP(ne3_d[:].tensor, b * PB * BC,
                                [[BC, 125], [1, BC]]))

            # warm the ACT table (sqrt set) while loads run, so band 0's
            # cav does not eat the 1.3us table-load latency
            t_sc = pool.tile([1, 2], bf16, tag="scw")
            nc.gpsimd.memset(t_sc[:], 1.0)
            nc.scalar.activation(out=t_sc[0:1, 0:1], in_=t_sc[0:1, 1:2],
                                 func=Act.Sqrt)

            # PSUM: two rotating per-band gradient tiles (2 banks each,
            # col-halves at 512-f32 offsets) + 4-slot velocity tile so the
            # PE runs ahead of the ACT cav consumer.  8 banks total.
            ps_g0 = ppool.tile([125, 2, 512], f32, tag="psg0")
            ps_g1 = ppool.tile([125, 2, 512], f32, tag="psg1")
            ps_gs = [ps_g0, ps_g1, ps_g0]
            ps_v = ppool.tile([125, 4, 512], f32, tag="psv")

            w_ver = t_w[0:127, 0, 0:125]
            w_p1 = t_w[0:127, 1, 0:125]
            w_m1 = t_w[0:127, 2, 0:125]
            w_id = t_w[0:125, 3, 0:125]
            w_kvv = t_w[0:126, 4, 0:125]

            t_cav = pool.tile([125, NB, BC], bf16, tag="cav")

            mm = nc.tensor.matmul
            for b in range(NB):
                for h in range(2):
                    c0 = h * HC
                    og = ps_gs[b][0:125, h, 0:HC]
                    mm(out=og, lhsT=w_ver,
                       rhs=t_ne[0:127, b, c0 + 1:c0 + HC + 1],
                       start=True, stop=False)
                    mm(out=og, lhsT=w_p1,
                       rhs=t_ne[0:127, b, c0 + 2:c0 + HC + 2],
                       start=False, stop=False)
                    mm(out=og, lhsT=w_m1,
                       rhs=t_ne[0:127, b, c0:c0 + HC],
                       start=False, stop=False)
                    mm(out=og, lhsT=w_id,
                       rhs=t_geo[0:125, b, c0:c0 + HC],
                       start=False, stop=True)
                    ov = ps_v[0:125, (2 * b + h) % 4, 0:HC]
                    mm(out=ov, lhsT=w_kvv,
                       rhs=t_vv[0:126, b, c0:c0 + HC],
                       start=True, stop=False)
                    mm(out=ov, lhsT=w_id,
                       rhs=t_vh[0:125, b, c0:c0 + HC],
                       start=False, stop=False)
                    mm(out=ov, lhsT=w_id,
                       rhs=t_vh[0:125, b, c0 + 1:c0 + HC + 1],
                       start=False, stop=True)
                # cav = |psum_v|*SNE/SV8 (= SNE*cav/SCALE); slot pairs
                # (0,1)/(2,3) rotate per band
                s0 = (2 * b) % 4
                nc.scalar.activation(
                    out=t_cav[0:125, b, :],
                    in_=ps_v[0:125, s0:s0 + 2, 0:HC],
                    func=Act.Abs, scale=float(SNE / SV8))

            def T(tag, dt=bf16):
                return pool.tile([125, BC], dt, tag=tag, name=tag)

            # Per-band pipelined tail; plain tensor_tensor (bf16 gets the
            # DVE 2x mode).  Front stages (num..ncsc) issue for all bands
            # first so the last band's chain is not starved; nonlinear tails
            # follow band-major.  The flux sign comes from num = dis*G
            # (dis > 0) via a min/max clip instead of an ACT Sign op.
            S_NCS = PHI08 / SNE ** 0.4
            Gs, diss, cavs, ncscs, sgts = [], [], [], [], []
            for c in range(NB):
                Gc = ps_gs[c][0:125, :, 0:HC]
                disc = t_dis[:, c, :]
                cavc = t_cav[:, c, :]
                Gs.append(Gc); diss.append(disc); cavs.append(cavc)

                num = T(f"num{c}")
                nc.vector.tensor_tensor(out=num[:], in0=disc,
                                        in1=Gc, op=Alu.mult)
                numer = T(f"numer{c}")
                nc.gpsimd.tensor_tensor(out=numer[:], in0=num[:],
                                        in1=cavc, op=Alu.add)
                den = T(f"den{c}")
                nc.vector.tensor_tensor(out=den[:], in0=t_ne3[:, c, :],
                                        in1=cavc, op=Alu.add)
                rec = T(f"rec{c}")
                nc.vector.reciprocal(out=rec[:], in_=den[:])
                ncs = T(f"ncs{c}")
                nc.vector.tensor_tensor(out=ncs[:], in0=numer[:],
                                        in1=rec[:], op=Alu.mult)
                ncsc = T(f"ncsc{c}")
                nc.vector.tensor_scalar(out=ncsc[:], in0=ncs[:],
                                        scalar1=float(S_NCS),
                                        scalar2=float(NCS_CLAMP * S_NCS),
                                        op0=Alu.mult, op1=Alu.max)
                ncscs.append(ncsc)
                sg1 = T(f"sg1{c}")
                nc.vector.tensor_scalar(out=sg1[:], in0=num[:],
                                        scalar1=1e30, scalar2=1.0,
                                        op0=Alu.mult, op1=Alu.min)
                sgt = T(f"sgt{c}")
                nc.vector.tensor_scalar_max(out=sgt[:], in0=sg1[:],
                                            scalar1=-1.0)
                sgts.append(sgt)

            for c in range(NB):
                ncsc = ncscs[c]
                u1 = T(f"u1{c}")
                nc.scalar.activation(out=u1[:], in_=ncsc[:], func=Act.Sqrt)
                u2 = T(f"u2{c}")
                nc.scalar.activation(out=u2[:], in_=ncsc[:],
                                     func=Act.Square)
                u3 = T(f"u3{c}")
                nc.vector.tensor_tensor(out=u3[:], in0=u1[:],
                                        in1=u2[:], op=Alu.mult)
                ab = T(f"ab{c}")
                nc.scalar.activation(out=ab[:], in_=Gs[c], func=Act.Abs)
                u4 = T(f"u4{c}")
                nc.vector.tensor_tensor(out=u4[:], in0=u3[:],
                                        in1=ab[:], op=Alu.mult)
                fm = T(f"fm{c}")
                nc.scalar.activation(out=fm[:], in_=u4[:], func=Act.Sqrt)
                f2 = T(f"f2{c}")
                nc.vector.tensor_tensor(out=f2[:], in0=fm[:],
                                        in1=sgts[c], op=Alu.mult)
                res = T(f"res{c}")
                nc.gpsimd.tensor_tensor(out=res[:], in0=diss[c],
                                        in1=f2[:], op=Alu.subtract)
                nc.sync.dma_start(
                    out=bass.AP(out_d[:].tensor, c * PB * BC,
                                [[BC, 125], [1, BC]]),
                    in_=res[:])

    # Compute instructions may carry at most ONE sync wait on TRN2; this
    # pass splits multi-wait instructions into EventSemaphore pairs (which
    # legally carry two).
    import bass_rust as _br
    _br.generate_event_semaphores(nc)
    return nc


def _raster_ok(head, tail):
    """Cheap check that head/tail are the expected raster links."""
    n_h = NROWS * (NCOLS - 1)
    n_links = n_h + (NROWS - 1) * NCOLS
    if head.shape[0] != n_links or tail.shape[0] != n_links:
        return False
    ids = np.arange(NROWS * NCOLS, dtype=np.int64).reshape(NROWS, NCOLS)
    s = slice(None, None, 9973)
    h_h = ids[:, 1:].ravel()
    h_t = ids[:, :-1].ravel()
    v_h = ids[1:, :].ravel()
    v_t = ids[:-1, :].ravel()
    return (
        np.array_equal(head[:n_h][s], h_h[s])
        and np.array_equal(tail[:n_h][s], h_t[s])
        and np.array_equal(head[n_h:][s], v_h[s])
        and np.array_equal(tail[n_h:][s], v_t[s])
        and head[n_h - 1] == h_h[-1]
        and tail[-1] == v_t[-1]
    )


def _fallback_numpy(effective_pressure, discharge, geometric_gradient,
                    overburden_pressure, sliding_velocity, link_length,
                    head, tail, status_at_node):
    """Exact general-graph port of the reference (host math, insurance only)."""
    n = effective_pressure.shape[0]
    head = head.astype(np.int64)
    tail = tail.astype(np.int64)

    def seg(v):
        return (np.bincount(head, weights=v, minlength=n)
                + np.bincount(tail, weights=v, minlength=n))

    cnt = np.maximum(seg(np.ones_like(link_length, dtype=np.float64)), 1.0)
    ne = np.where(status_at_node != 0, overburden_pressure,
                  effective_pressure).astype(np.float64)
    grad_l = (ne[head] - ne[tail]) / link_length
    grad = seg(grad_l) / cnt + geometric_gradient
    cav = np.abs(seg(sliding_velocity / SEC_PER_A) / cnt) * STEP_HEIGHT
    cs = ((OPENING_COEFF * discharge * grad + cav)
          / (cav / SCALE_CUTOFF + CLOSURE_COEFF * ne ** N_EXP))
    cs = np.where(cs < 1e-6, 1e-6, cs)
    res = (discharge - OPENING_COEFF * cs ** FLOW_EXP
           * np.abs(grad) ** (-0.5) * grad)
    return res.astype(np.float32)


def _build_weights():
    """Packed PE shift matrices [127, 5, 128] fp8 (lhsT layout [K, M])."""
    w = np.zeros((127, 5, 128), np.float32)
    j = np.arange(125)
    w[j + 2, 0, j] = 1.0   # Wver: +S
    w[j, 0, j] = -1.0      # Wver: -N
    w[j + 1, 1, j] = 1.0   # Wp1:  +E (rhs pre-shifted)
    w[j + 1, 2, j] = -1.0  # Wm1:  -W
    w[j, 3, j] = 1.0       # I125 (geo / vh), rhs at partitions 0..124
    w[j, 4, j] = 1.0       # Kvv row r
    w[j + 1, 4, j] = 1.0   # Kvv row r+1
    return w.reshape(127, 5 * 128).astype(FP8)


def _make_in_maps(effective_pressure, discharge, geometric_gradient,
                  overburden_pressure, sliding_velocity, status_at_node):
    nh = NROWS * (NCOLS - 1)
    eff2 = np.asarray(effective_pressure, np.float32).reshape(NROWS, NCOLS)
    over2 = np.asarray(overburden_pressure, np.float32).reshape(NROWS, NCOLS)
    stat2 = np.asarray(status_at_node, np.int32).reshape(NROWS, NCOLS)
    dis2 = np.asarray(discharge, np.float32).reshape(NROWS, NCOLS)
    geo2 = np.asarray(geometric_gradient, np.float32).reshape(NROWS, NCOLS)
    sv = np.asarray(sliding_velocity, np.float32)

    ne = np.where(stat2 != 0, over2, eff2)
    nes = ne * np.float32(AK * SNE)
    nep = np.pad(nes, 1, mode="edge").astype(FP8)
    ne3 = ((ne * np.float32(AK)).astype(np.float64) ** 3
           * C3 * SNE).astype(np.float32).astype(BF16)
    geos = (geo2 * np.float32(KAPPA * SNE)).astype(FP8)
    vhp = np.zeros((NROWS, NCOLS + 1), np.float32)
    vhp[:, 1:NCOLS] = sv[:nh].reshape(NROWS, NCOLS - 1)
    vhp = (vhp * np.float32(BETA * SV8)).astype(FP8)
    vvp = np.zeros((NROWS + 1, NCOLS), np.float32)
    vvp[1:NROWS, :] = sv[nh:].reshape(NROWS - 1, NCOLS)
    vvp = (vvp * np.float32(BETA * SV8)).astype(FP8)
    dis2 = dis2.astype(BF16)
    wf = _build_weights()

    in_maps = []
    for i in range(CI):
        for j in range(CJ):
            r0, c0 = BR * i, BC * j
            m = {
                "ne": np.ascontiguousarray(
                    nep[r0:r0 + BR + 2, c0:c0 + WNE]),
                "ne3": np.ascontiguousarray(
                    ne3[r0:r0 + BR, c0:c0 + BC]),
                "dis": np.ascontiguousarray(dis2[r0:r0 + BR, c0:c0 + BC]),
                "geo": np.ascontiguousarray(geos[r0:r0 + BR, c0:c0 + BC]),
                "vh": np.ascontiguousarray(
                    vhp[r0:r0 + BR, c0:c0 + BC + 1]),
                "vv": np.ascontiguousarray(
                    vvp[r0:r0 + BR + 1, c0:c0 + BC]),
                "wf": wf,
            }
            in_maps.append(m)
    return in_maps


def _frame_fix(full, eff2, over2, stat2, dis2, geo2, sv):
    """Exact host residual for the global frame (link_count != 4)."""
    nh = NROWS * (NCOLS - 1)
    ne = np.where(stat2 != 0, over2, eff2).astype(np.float64)
    nep = np.pad(ne, 1, mode="edge")
    vhp = np.zeros((NROWS, NCOLS + 1), np.float64)
    vhp[:, 1:NCOLS] = sv[:nh].reshape(NROWS, NCOLS - 1)
    vvp = np.zeros((NROWS + 2, NCOLS), np.float64)
    vvp[1:NROWS, :] = sv[nh:].reshape(NROWS - 1, NCOLS)

    r_idx = np.arange(NROWS)
    c_idx = np.arange(NCOLS)
    cnt2 = (4.0 - (r_idx[:, None] == 0) - (r_idx[:, None] == NROWS - 1)
            - (c_idx[None, :] == 0) - (c_idx[None, :] == NCOLS - 1))

    def strip(rs, cs):
        r = r_idx[rs][:, None]
        c = c_idx[cs][None, :]
        cnt = cnt2[rs][:, cs]
        sumg = (nep[r + 1, c + 2] - nep[r + 1, c]
                + nep[r + 2, c + 1] - nep[r, c + 1]) / DX
        grad = sumg / cnt + geo2[rs][:, cs]
        cav = (np.abs(vhp[r, c] + vhp[r, c + 1]
                      + vvp[r, c] + vvp[r + 1, c]) / cnt
               * (STEP_HEIGHT / SEC_PER_A))
        nel = ne[rs][:, cs]
        disl = dis2[rs][:, cs]
        cs_ = ((OPENING_COEFF * disl * grad + cav)
               / (cav / SCALE_CUTOFF + CLOSURE_COEFF * nel ** N_EXP))
        cs_ = np.where(cs_ < 1e-6, 1e-6, cs_)
        res = (disl - OPENING_COEFF * cs_ ** FLOW_EXP
               * np.abs(grad) ** (-0.5) * grad)
        full[rs][:, cs] = res.astype(np.float32)
        return res.astype(np.float32)

    allc = slice(None)
    full[0, :] = strip(slice(0, 1), allc)[0]
    full[NROWS - 1, :] = strip(slice(NROWS - 1, NROWS), allc)[0]
    full[:, 0] = strip(allc, slice(0, 1))[:, 0]
    full[:, NCOLS - 1] = strip(allc, slice(NCOLS - 1, NCOLS))[:, 0]


def run_on_cores(in_maps, trace=False):
    from concourse.bass_utils import run_bass_kernel_spmd

    if "nc" not in _NC_CACHE:
        _NC_CACHE["nc"] = _build_nc()
    return run_bass_kernel_spmd(
        _NC_CACHE["nc"], in_maps, list(range(8)), trace=trace)


def kernel(effective_pressure, discharge, geometric_gradient,
           overburden_pressure, sliding_velocity, link_length,
           head, tail, status_at_node):
    effective_pressure = np.asarray(effective_pressure)
    link_length = np.asarray(link_length)
    head = np.asarray(head)
    tail = np.asarray(tail)
    ll0 = float(link_length[0]) if link_length.size else 100.0
    if (not _raster_ok(head, tail) or abs(ll0 - 100.0) > 1e-6
            or not np.all(link_length[::9973] == ll0)):
        return _fallback_numpy(
            np.asarray(effective_pressure), np.asarray(discharge),
            np.asarray(geometric_gradient), np.asarray(overburden_pressure),
            np.asarray(sliding_velocity), link_length, head, tail,
            np.asarray(status_at_node))

    in_maps = _make_in_maps(effective_pressure, discharge,
                            geometric_gradient, overburden_pressure,
                            sliding_velocity, status_at_node)
    results = run_on_cores(in_maps).results

    full = np.empty((NROWS, NCOLS), np.float32)
    k = 0
    for i in range(CI):
        for j in range(CJ):
            full[BR * i:BR * (i + 1), BC * j:BC * (j + 1)] = (
                results[k]["res"].astype(np.float32))
            k += 1

    _frame_fix(
        full,
        np.asarray(effective_pressure, np.float32).reshape(NROWS, NCOLS),
        np.asarray(overburden_pressure, np.float32).reshape(NROWS, NCOLS),
        np.asarray(status_at_node, np.int32).reshape(NROWS, NCOLS),
        np.asarray(discharge, np.float32).reshape(NROWS, NCOLS),
        np.asarray(geometric_gradient, np.float32).reshape(NROWS, NCOLS),
        np.asarray(sliding_velocity, np.float32))
    return full.ravel()


# revision 37
# speedup vs baseline: 1.1247x; 1.1247x over previous
"""Trainium2 Bass kernel for ConduitHydrology (GNN message passing on a
1500x1500 raster grid).

The mesh is the fixed 2D raster built by the reference: every segment_sum
over head/tail collapses into a 5-point stencil.  The residual is
  res = dis - flux,  flux = OPEN*cs^1.25*|g|^-0.5*g  (|flux| <~ 2e-4)
so the residual is dominated by `dis`; every other input only feeds the
tiny flux term, which lets the whole stencil+conduit pipeline run in bf16
with enormous margin vs the 2e-2 tolerance (dis itself stays f32).

Sharding: 2x4 grid of cores, each owns a 750x375 node block, split into
6 row-bands of 125 rows.  All cross-partition (vertical) stencil work is
done on the otherwise-idle PE as shift-matrix matmuls accumulating in
PSUM (gradient: Wver*neC + Wp1*neE + Wm1*neW + I*geo; velocity:
Kvv*vv + I*vhW + I*vhC), with constants folded into host-scaled inputs:
  A   = ne * (kappa/(4L))        [ne = where(stat, over, eff), edge-pad]
  G   = psum_g = stencil(A)+geo*kappa = kappa*gradient, kappa=OPEN/SCALE
  C   = |psum_v| = cav/SCALE     [vh, vv scaled by STEP/(4*SEC*SCALE)]
  ncs = (dis*G + C)/(C + c3*A^3) = cs/SCALE,   c3 = CLOSURE/(kappa/(4L))^3
  flux= ncs_c^1.25 * G * 1/sqrt(s*|G|),        s = Phi^-2,
        Phi = OPEN*SCALE^1.25/sqrt(kappa)
Global frame nodes (link_count != 4) are fixed up exactly on the host
(5996 of 2.25M nodes).
"""

import sys

import numpy as np

if "/opt/trn_rl_repo" not in sys.path:
    sys.path.insert(0, "/opt/trn_rl_repo")

import ml_dtypes

BF16 = ml_dtypes.bfloat16
FP8 = (ml_dtypes.float8_e4m3fn if hasattr(ml_dtypes, "float8_e4m3fn")
       else ml_dtypes.float8_e4m3)

# ---- problem constants (from the reference model) ----
NROWS, NCOLS = 1500, 1500
OPENING_COEFF = 1.3455e-09
CLOSURE_COEFF = 7.11e-24
FLOW_EXP = 1.25
STEP_HEIGHT = 0.03
SCALE_CUTOFF = 5.74
N_EXP = 3
SEC_PER_A = 31556926.0
DX = 100.0

# ---- folded constants ----
ALPHA = 1.0 / (4.0 * DX)                     # 1/(L*cnt), interior cnt=4
KAPPA = OPENING_COEFF / SCALE_CUTOFF         # gradient scale
AK = ALPHA * KAPPA                           # ne scale
BETA = STEP_HEIGHT / (4.0 * SEC_PER_A * SCALE_CUTOFF)  # velocity scale
C3 = CLOSURE_COEFF / (AK ** 3)               # conduit denominator scale
PHI = OPENING_COEFF * SCALE_CUTOFF ** 1.25 / np.sqrt(KAPPA)
S_ARS = 1.0 / (PHI * PHI)                    # Abs_reciprocal_sqrt scale
NCS_CLAMP = 1e-6 / SCALE_CUTOFF              # conduit-size clamp on ncs
PHI08 = PHI ** 0.8                           # folds Phi^2 into ncs^2.5
SNE = 2.0 ** 21                              # fp8 scale for ne/geo/ne3
SV8 = 2.0 ** 26                              # fp8 scale for vh/vv

# ---- sharding geometry: 4x2 grid of cores ----
# 750-wide rows keep fp8 DMA descriptors >= 512B (full DMA rate)
CI, CJ = 4, 2
BR, BC = NROWS // CI, NCOLS // CJ            # 375 x 750 per core
NB = 3                                       # row bands per core
PB = BR // NB                                # 125 rows per band
HC = BC // 2                                 # 375: matmul col-half (PSUM bank)
WNE = BC + 2                                 # 752 ne cols (with halo)

_NC_CACHE = {}


def _patch_tile_drain():
    """The end-of-kernel Drain that Tile emits carries one sync-wait per
    outstanding semaphore; this stack's codegen rejects instructions with
    more than a handful of waits.  Split the collector into one NOP per
    proc, each carrying exactly one wait (the sync queue is in-order, so
    this is equivalent)."""
    from concourse import tile as _tile
    from concourse.vector_clock import ScopedClock, VectorClock

    if getattr(_tile.TileContext, "_drain_patched", False):
        return

    def _drain_and_barrier(self, tick_clock, wait_clock):
        gc = tick_clock.global_clock
        n = len(gc)
        for proc in range(n):
            t = gc[proc]
            if t <= 0:
                continue
            nop = self.nc.sync.nop()
            vc = VectorClock([0] * n)
            vc.require_at_least(proc, t)
            wait_clock.add_sem_waits(nop.ins, ScopedClock({None: vc}))
        self.nc.sync.drain()
        self.nc.all_engine_barrier()
        assert self.sems is not None
        popped = self.nc._tile_sem_poison_stack.pop()
        assert popped is self._sem_poison
        self.nc.clear_and_free_semaphores(list(self.sems.allocated().values()))
        self.nc.all_engine_barrier()

    _tile.TileContext._drain_and_barrier = _drain_and_barrier
    _tile.TileContext._drain_patched = True


def _build_nc():
    import concourse.bass as bass
    import concourse.mybir as mybir
    from concourse import bacc
    from concourse.tile import TileContext

    _patch_tile_drain()

    f32 = mybir.dt.float32
    bf16 = mybir.dt.bfloat16
    f8 = mybir.dt.float8e4
    Alu = mybir.AluOpType
    Act = mybir.ActivationFunctionType

    nc = bass.Bass()

    ne_d = nc.dram_tensor("ne", [BR + 2, WNE], f8, kind="ExternalInput")
    ne3_d = nc.dram_tensor("ne3", [BR, BC], bf16, kind="ExternalInput")
    dis_d = nc.dram_tensor("dis", [BR, BC], bf16, kind="ExternalInput")
    geo_d = nc.dram_tensor("geo", [BR, BC], f8, kind="ExternalInput")
    vh_d = nc.dram_tensor("vh", [BR, BC + 1], f8, kind="ExternalInput")
    vv_d = nc.dram_tensor("vv", [BR + 1, BC], f8, kind="ExternalInput")
    wf_d = nc.dram_tensor("wf", [127, 5 * 128], f8, kind="ExternalInput")
    out_d = nc.dram_tensor("res", [BR, BC], bf16, kind="ExternalOutput")

    with TileContext(nc) as tc:
        with tc.tile_pool(name="p", bufs=1) as pool, \
                tc.psum_pool(name="pp", bufs=1) as ppool, \
                nc.allow_low_precision(
                    reason="flux term is <1e-4 of the residual; bf16/fp8 "
                    "error is far inside the 2e-2 tolerance"):
            t_ne = pool.tile([127, NB, WNE], f8, tag="ne")
            t_ne3 = pool.tile([125, NB, BC], bf16, tag="ne3")
            t_dis = pool.tile([125, NB, BC], bf16, tag="dis")
            t_geo = pool.tile([125, NB, BC], f8, tag="geo")
            t_vh = pool.tile([125, NB, BC + 1], f8, tag="vh")
            t_vv = pool.tile([126, NB, BC], f8, tag="vv")
            t_w = pool.tile([127, 5, 128], f8, tag="wf")

            # loads; ne/geo first so the PE gradient groups start early,
            # dis/ne3 split per band so band 0's conduit chain starts early
            nc.sync.dma_start(out=t_w[:], in_=wf_d[:])
            nc.sync.dma_start(
                out=t_ne[:],
                in_=bass.AP(ne_d[:].tensor, 0,
                            [[WNE, 127], [PB * WNE, NB], [1, WNE]]))
            nc.sync.dma_start(
                out=t_geo[:],
                in_=bass.AP(geo_d[:].tensor, 0,
                            [[BC, 125], [PB * BC, NB], [1, BC]]))
            nc.sync.dma_start(
                out=t_vh[:],
                in_=bass.AP(vh_d[:].tensor, 0,
                            [[BC + 1, 125], [PB * (BC + 1), NB],
                             [1, BC + 1]]))
            nc.sync.dma_start(
                out=t_vv[:],
                in_=bass.AP(vv_d[:].tensor, 0,
                            [[BC, 126], [PB * BC, NB], [1, BC]]))
            for b in range(NB):
                nc.sync.dma_start(
                    out=t_dis[:, b, :],
                    in_=bass.AP(dis_d[:].tensor, b * PB * BC,
                                [[BC, 125], [1, BC]]))
                nc.sync.dma_start(
                    out=t_ne3[:, b, :],
                    in_=bass.AP(ne3_d[:].tensor, b * PB * BC,
                                [[BC, 125], [1, BC]]))

            # warm the ACT table (sqrt set) while loads run, so band 0's
            # cav does not eat the 1.3us table-load latency
            t_sc = pool.tile([1, 2], bf16, tag="scw")
            nc.gpsimd.memset(t_sc[:], 1.0)
            nc.scalar.activation(out=t_sc[0:1, 0:1], in_=t_sc[0:1, 1:2],
                                 func=Act.Sqrt)

            # PSUM: two rotating per-band gradient tiles (2 banks each,
            # col-halves at 512-f32 offsets) + 4-slot velocity tile so the
            # PE runs ahead of the ACT cav consumer.  8 banks total.
            ps_g0 = ppool.tile([125, 2, 512], f32, tag="psg0")
            ps_g1 = ppool.tile([125, 2, 512], f32, tag="psg1")
            ps_gs = [ps_g0, ps_g1, ps_g0]
            ps_v = ppool.tile([125, 4, 512], f32, tag="psv")

            w_ver = t_w[0:127, 0, 0:125]
            w_p1 = t_w[0:127, 1, 0:125]
            w_m1 = t_w[0:127, 2, 0:125]
            w_id = t_w[0:125, 3, 0:125]
            w_kvv = t_w[0:126, 4, 0:125]

            t_cav = pool.tile([125, NB, BC], bf16, tag="cav")

            mm = nc.tensor.matmul
            for b in range(NB):
                for h in range(2):
                    c0 = h * HC
                    og = ps_gs[b][0:125, h, 0:HC]
                    mm(out=og, lhsT=w_ver,
                       rhs=t_ne[0:127, b, c0 + 1:c0 + HC + 1],
                       start=True, stop=False)
                    mm(out=og, lhsT=w_p1,
                       rhs=t_ne[0:127, b, c0 + 2:c0 + HC + 2],
                       start=False, stop=False)
                    mm(out=og, lhsT=w_m1,
                       rhs=t_ne[0:127, b, c0:c0 + HC],
                       start=False, stop=False)
                    mm(out=og, lhsT=w_id,
                       rhs=t_geo[0:125, b, c0:c0 + HC],
                       start=False, stop=True)
                    ov = ps_v[0:125, (2 * b + h) % 4, 0:HC]
                    mm(out=ov, lhsT=w_kvv,
                       rhs=t_vv[0:126, b, c0:c0 + HC],
                       start=True, stop=False)
                    mm(out=ov, lhsT=w_id,
                       rhs=t_vh[0:125, b, c0:c0 + HC],
                       start=False, stop=False)
                    mm(out=ov, lhsT=w_id,
                       rhs=t_vh[0:125, b, c0 + 1:c0 + HC + 1],
                       start=False, stop=True)
                # cav = |psum_v|*SNE/SV8 (= SNE*cav/SCALE); slot pairs
                # (0,1)/(2,3) rotate per band
                s0 = (2 * b) % 4
                nc.scalar.activation(
                    out=t_cav[0:125, b, :],
                    in_=ps_v[0:125, s0:s0 + 2, 0:HC],
                    func=Act.Abs, scale=float(SNE / SV8))

            def T(tag, dt=bf16):
                return pool.tile([125, BC], dt, tag=tag, name=tag)

            # Per-band pipelined tail; plain tensor_tensor (bf16 gets the
            # DVE 2x mode).  Front stages (num..ncsc) issue for all bands
            # first so the last band's chain is not starved; nonlinear tails
            # follow band-major.  The flux sign comes from num = dis*G
            # (dis > 0) via a min/max clip instead of an ACT Sign op.
            S_NCS = PHI08 / SNE ** 0.4
            Gs, diss, cavs, ncscs, sgts = [], [], [], [], []
            for c in range(NB):
                Gc = ps_gs[c][0:125, :, 0:HC]
                disc = t_dis[:, c, :]
                cavc = t_cav[:, c, :]
                Gs.append(Gc); diss.append(disc); cavs.append(cavc)

                num = T(f"num{c}")
                nc.vector.tensor_tensor(out=num[:], in0=disc,
                                        in1=Gc, op=Alu.mult)
                numer = T(f"numer{c}")
                nc.vector.tensor_tensor(out=numer[:], in0=num[:],
                                        in1=cavc, op=Alu.add)
                den = T(f"den{c}")
                nc.gpsimd.tensor_tensor(out=den[:], in0=t_ne3[:, c, :],
                                        in1=cavc, op=Alu.add)
                rec = T(f"rec{c}")
                nc.vector.reciprocal(out=rec[:], in_=den[:])
                ncs = T(f"ncs{c}")
                nc.vector.tensor_tensor(out=ncs[:], in0=numer[:],
                                        in1=rec[:], op=Alu.mult)
                ncsc = T(f"ncsc{c}")
                nc.vector.tensor_scalar(out=ncsc[:], in0=ncs[:],
                                        scalar1=float(S_NCS),
                                        scalar2=float(NCS_CLAMP * S_NCS),
                                        op0=Alu.mult, op1=Alu.max)
                ncscs.append(ncsc)
                sg1 = T(f"sg1{c}")
                nc.vector.tensor_scalar(out=sg1[:], in0=num[:],
                                        scalar1=1e30, scalar2=1.0,
                                        op0=Alu.mult, op1=Alu.min)
                sgt = T(f"sgt{c}")
                nc.vector.tensor_scalar_max(out=sgt[:], in0=sg1[:],
                                            scalar1=-1.0)
                sgts.append(sgt)

            for c in range(NB):
                ncsc = ncscs[c]
                u1 = T(f"u1{c}")
                nc.scalar.activation(out=u1[:], in_=ncsc[:], func=Act.Sqrt)
                u2 = T(f"u2{c}")
                nc.scalar.activation(out=u2[:], in_=ncsc[:],
                                     func=Act.Square)
                u3 = T(f"u3{c}")
                nc.vector.tensor_tensor(out=u3[:], in0=u1[:],
                                        in1=u2[:], op=Alu.mult)
                ab = T(f"ab{c}")
                nc.scalar.activation(out=ab[:], in_=Gs[c], func=Act.Abs)
                u4 = T(f"u4{c}")
                nc.vector.tensor_tensor(out=u4[:], in0=u3[:],
                                        in1=ab[:], op=Alu.mult)
                fm = T(f"fm{c}")
                nc.scalar.activation(out=fm[:], in_=u4[:], func=Act.Sqrt)
                f2 = T(f"f2{c}")
                nc.vector.tensor_tensor(out=f2[:], in0=fm[:],
                                        in1=sgts[c], op=Alu.mult)
                res = T(f"res{c}")
                nc.vector.tensor_tensor(out=res[:], in0=diss[c],
                                        in1=f2[:], op=Alu.subtract)
                nc.sync.dma_start(
                    out=bass.AP(out_d[:].tensor, c * PB * BC,
                                [[BC, 125], [1, BC]]),
                    in_=res[:])

    # Compute instructions may carry at most ONE sync wait on TRN2; this
    # pass splits multi-wait instructions into EventSemaphore pairs (which
    # legally carry two).
    import bass_rust as _br
    _br.generate_event_semaphores(nc)
    return nc


def _raster_ok(head, tail):
    """Cheap check that head/tail are the expected raster links."""
    n_h = NROWS * (NCOLS - 1)
    n_links = n_h + (NROWS - 1) * NCOLS
    if head.shape[0] != n_links or tail.shape[0] != n_links:
        return False
    ids = np.arange(NROWS * NCOLS, dtype=np.int64).reshape(NROWS, NCOLS)
    s = slice(None, None, 9973)
    h_h = ids[:, 1:].ravel()
    h_t = ids[:, :-1].ravel()
    v_h = ids[1:, :].ravel()
    v_t = ids[:-1, :].ravel()
    return (
        np.array_equal(head[:n_h][s], h_h[s])
        and np.array_equal(tail[:n_h][s], h_t[s])
        and np.array_equal(head[n_h:][s], v_h[s])
        and np.array_equal(tail[n_h:][s], v_t[s])
        and head[n_h - 1] == h_h[-1]
        and tail[-1] == v_t[-1]
    )


def _fallback_numpy(effective_pressure, discharge, geometric_gradient,
                    overburden_pressure, sliding_velocity, link_length,
                    head, tail, status_at_node):
    """Exact general-graph port of the reference (host math, insurance only)."""
    n = effective_pressure.shape[0]
    head = head.astype(np.int64)
    tail = tail.astype(np.int64)

    def seg(v):
        return (np.bincount(head, weights=v, minlength=n)
                + np.bincount(tail, weights=v, minlength=n))

    cnt = np.maximum(seg(np.ones_like(link_length, dtype=np.float64)), 1.0)
    ne = np.where(status_at_node != 0, overburden_pressure,
                  effective_pressure).astype(np.float64)
    grad_l = (ne[head] - ne[tail]) / link_length
    grad = seg(grad_l) / cnt + geometric_gradient
    cav = np.abs(seg(sliding_velocity / SEC_PER_A) / cnt) * STEP_HEIGHT
    cs = ((OPENING_COEFF * discharge * grad + cav)
          / (cav / SCALE_CUTOFF + CLOSURE_COEFF * ne ** N_EXP))
    cs = np.where(cs < 1e-6, 1e-6, cs)
    res = (discharge - OPENING_COEFF * cs ** FLOW_EXP
           * np.abs(grad) ** (-0.5) * grad)
    return res.astype(np.float32)


def _build_weights():
    """Packed PE shift matrices [127, 5, 128] fp8 (lhsT layout [K, M])."""
    w = np.zeros((127, 5, 128), np.float32)
    j = np.arange(125)
    w[j + 2, 0, j] = 1.0   # Wver: +S
    w[j, 0, j] = -1.0      # Wver: -N
    w[j + 1, 1, j] = 1.0   # Wp1:  +E (rhs pre-shifted)
    w[j + 1, 2, j] = -1.0  # Wm1:  -W
    w[j, 3, j] = 1.0       # I125 (geo / vh), rhs at partitions 0..124
    w[j, 4, j] = 1.0       # Kvv row r
    w[j + 1, 4, j] = 1.0   # Kvv row r+1
    return w.reshape(127, 5 * 128).astype(FP8)


def _make_in_maps(effective_pressure, discharge, geometric_gradient,
                  overburden_pressure, sliding_velocity, status_at_node):
    nh = NROWS * (NCOLS - 1)
    eff2 = np.asarray(effective_pressure, np.float32).reshape(NROWS, NCOLS)
    over2 = np.asarray(overburden_pressure, np.float32).reshape(NROWS, NCOLS)
    stat2 = np.asarray(status_at_node, np.int32).reshape(NROWS, NCOLS)
    dis2 = np.asarray(discharge, np.float32).reshape(NROWS, NCOLS)
    geo2 = np.asarray(geometric_gradient, np.float32).reshape(NROWS, NCOLS)
    sv = np.asarray(sliding_velocity, np.float32)

    ne = np.where(stat2 != 0, over2, eff2)
    nes = ne * np.float32(AK * SNE)
    nep = np.pad(nes, 1, mode="edge").astype(FP8)
    ne3 = ((ne * np.float32(AK)).astype(np.float64) ** 3
           * C3 * SNE).astype(np.float32).astype(BF16)
    geos = (geo2 * np.float32(KAPPA * SNE)).astype(FP8)
    vhp = np.zeros((NROWS, NCOLS + 1), np.float32)
    vhp[:, 1:NCOLS] = sv[:nh].reshape(NROWS, NCOLS - 1)
    vhp = (vhp * np.float32(BETA * SV8)).astype(FP8)
    vvp = np.zeros((NROWS + 1, NCOLS), np.float32)
    vvp[1:NROWS, :] = sv[nh:].reshape(NROWS - 1, NCOLS)
    vvp = (vvp * np.float32(BETA * SV8)).astype(FP8)
    dis2 = dis2.astype(BF16)
    wf = _build_weights()

    in_maps = []
    for i in range(CI):
        for j in range(CJ):
            r0, c0 = BR * i, BC * j
            m = {
                "ne": np.ascontiguousarray(
                    nep[r0:r0 + BR + 2, c0:c0 + WNE]),
                "ne3": np.ascontiguousarray(
                    ne3[r0:r0 + BR, c0:c0 + BC]),
                "dis": np.ascontiguousarray(dis2[r0:r0 + BR, c0:c0 + BC]),
                "geo": np.ascontiguousarray(geos[r0:r0 + BR, c0:c0 + BC]),
                "vh": np.ascontiguousarray(
                    vhp[r0:r0 + BR, c0:c0 + BC + 1]),
                "vv": np.ascontiguousarray(
                    vvp[r0:r0 + BR + 1, c0:c0 + BC]),
                "wf": wf,
            }
            in_maps.append(m)
    return in_maps


def _frame_fix(full, eff2, over2, stat2, dis2, geo2, sv):
    """Exact host residual for the global frame (link_count != 4)."""
    nh = NROWS * (NCOLS - 1)
    ne = np.where(stat2 != 0, over2, eff2).astype(np.float64)
    nep = np.pad(ne, 1, mode="edge")
    vhp = np.zeros((NROWS, NCOLS + 1), np.float64)
    vhp[:, 1:NCOLS] = sv[:nh].reshape(NROWS, NCOLS - 1)
    vvp = np.zeros((NROWS + 2, NCOLS), np.float64)
    vvp[1:NROWS, :] = sv[nh:].reshape(NROWS - 1, NCOLS)

    r_idx = np.arange(NROWS)
    c_idx = np.arange(NCOLS)
    cnt2 = (4.0 - (r_idx[:, None] == 0) - (r_idx[:, None] == NROWS - 1)
            - (c_idx[None, :] == 0) - (c_idx[None, :] == NCOLS - 1))

    def strip(rs, cs):
        r = r_idx[rs][:, None]
        c = c_idx[cs][None, :]
        cnt = cnt2[rs][:, cs]
        sumg = (nep[r + 1, c + 2] - nep[r + 1, c]
                + nep[r + 2, c + 1] - nep[r, c + 1]) / DX
        grad = sumg / cnt + geo2[rs][:, cs]
        cav = (np.abs(vhp[r, c] + vhp[r, c + 1]
                      + vvp[r, c] + vvp[r + 1, c]) / cnt
               * (STEP_HEIGHT / SEC_PER_A))
        nel = ne[rs][:, cs]
        disl = dis2[rs][:, cs]
        cs_ = ((OPENING_COEFF * disl * grad + cav)
               / (cav / SCALE_CUTOFF + CLOSURE_COEFF * nel ** N_EXP))
        cs_ = np.where(cs_ < 1e-6, 1e-6, cs_)
        res = (disl - OPENING_COEFF * cs_ ** FLOW_EXP
               * np.abs(grad) ** (-0.5) * grad)
        full[rs][:, cs] = res.astype(np.float32)
        return res.astype(np.float32)

    allc = slice(None)
    full[0, :] = strip(slice(0, 1), allc)[0]
    full[NROWS - 1, :] = strip(slice(NROWS - 1, NROWS), allc)[0]
    full[:, 0] = strip(allc, slice(0, 1))[:, 0]
    full[:, NCOLS - 1] = strip(allc, slice(NCOLS - 1, NCOLS))[:, 0]


def run_on_cores(in_maps, trace=False):
    from concourse.bass_utils import run_bass_kernel_spmd

    if "nc" not in _NC_CACHE:
        _NC_CACHE["nc"] = _build_nc()
    return run_bass_kernel_spmd(
        _NC_CACHE["nc"], in_maps, list(range(8)), trace=trace)


def kernel(effective_pressure, discharge, geometric_gradient,
           overburden_pressure, sliding_velocity, link_length,
           head, tail, status_at_node):
    effective_pressure = np.asarray(effective_pressure)
    link_length = np.asarray(link_length)
    head = np.asarray(head)
    tail = np.asarray(tail)
    ll0 = float(link_length[0]) if link_length.size else 100.0
    if (not _raster_ok(head, tail) or abs(ll0 - 100.0) > 1e-6
            or not np.all(link_length[::9973] == ll0)):
        return _fallback_numpy(
            np.asarray(effective_pressure), np.asarray(discharge),
            np.asarray(geometric_gradient), np.asarray(overburden_pressure),
            np.asarray(sliding_velocity), link_length, head, tail,
            np.asarray(status_at_node))

    in_maps = _make_in_maps(effective_pressure, discharge,
                            geometric_gradient, overburden_pressure,
                            sliding_velocity, status_at_node)
    results = run_on_cores(in_maps).results

    full = np.empty((NROWS, NCOLS), np.float32)
    k = 0
    for i in range(CI):
        for j in range(CJ):
            full[BR * i:BR * (i + 1), BC * j:BC * (j + 1)] = (
                results[k]["res"].astype(np.float32))
            k += 1

    _frame_fix(
        full,
        np.asarray(effective_pressure, np.float32).reshape(NROWS, NCOLS),
        np.asarray(overburden_pressure, np.float32).reshape(NROWS, NCOLS),
        np.asarray(status_at_node, np.int32).reshape(NROWS, NCOLS),
        np.asarray(discharge, np.float32).reshape(NROWS, NCOLS),
        np.asarray(geometric_gradient, np.float32).reshape(NROWS, NCOLS),
        np.asarray(sliding_velocity, np.float32))
    return full.ravel()
